# revision 34
# baseline (speedup 1.0000x reference)
"""4-layer GCN on 8 Trainium2 NeuronCores (Bass/Tile).

Sharding (dst-ownership, per the hint): core c owns dst nodes
[c*P, (c+1)*P), P = N/8. Each layer:
  1. each core computes its support block  h_own @ W  on the PE (bf16),
  2. AllGather of the blocks builds the full support table
     [N, 128]bf16 in DRAM (halo exchange),
  3. per-edge src rows are fetched with dma_gather (256B rows, int16
     indices in 4 src-buckets of N/4 rows),
  4. segment-sum by dst runs on the PE as one-hot matmuls: an indicator
     [128 edges x 128 dst] tile is built per chunk on the DVE with a
     single fused tensor_scalar (is_equal then *w), then
     psum[64f, 128dst] += msgs.T @ indicator accumulates f-major,
  5. finalize: ACT relu(agg + b) emits h^T tiles that feed the next
     layer's support matmul directly (no transposes anywhere).
Layer 4 aggregates node-major (indicator as lhsT) so log_softmax
reduces along the free axis.

SPMD: one program runs on all 8 cores, so per-(tile,bucket) group
sizes are the max over cores, rounded up to 128 (pad slots gather row
0 with w=0).
"""

import os
from contextlib import ExitStack

import numpy as np
import ml_dtypes

import concourse.bass as bass
import concourse.bacc as bacc
import concourse.tile as tile
import concourse.mybir as mybir
from concourse.bass_utils import run_bass_kernel_spmd

BF16 = mybir.dt.bfloat16
F32 = mybir.dt.float32
I16 = mybir.dt.int16
BFNP = ml_dtypes.bfloat16


class Cfg:
    def __init__(self, N=100000, CORES=8, NF=128, NH=64, NCL=40, G=7):
        self.N, self.CORES, self.NF, self.NH, self.NCL, self.G = N, CORES, NF, NH, NCL, G
        assert N % CORES == 0 and N % 4 == 0
        self.P = N // CORES
        self.BUCKET = N // 4
        assert self.BUCKET <= 32767
        self.NTILE = -(-self.P // 128)
        self.tile_nt = [128] * (self.NTILE - 1) + [self.P - 128 * (self.NTILE - 1)]
        self.NST = -(-self.NTILE // G)
        self.FPAD = 128  # padded feature dim of the gather table (256B rows)


def _ru128(x):
    return (x + 127) // 128 * 128


def build_plan(cfg, edge_src, edge_dst, edge_w):
    """Host-side preprocessing. Returns (meta, per_core) where meta is
    core-independent program structure and per_core holds input arrays."""
    N, P, G = cfg.N, cfg.P, cfg.G
    src = np.asarray(edge_src, np.int64)
    dst = np.asarray(edge_dst, np.int64)
    w = np.asarray(edge_w, np.float32)

    owner = dst // P
    per_edge = []
    cnts = np.zeros((cfg.CORES, cfg.NTILE, 4), np.int64)
    for c in range(cfg.CORES):
        m = owner == c
        sc, dl, wc = src[m], dst[m] - c * P, w[m]
        t = dl >> 7
        b = sc // cfg.BUCKET
        cnts[c] = np.bincount(t * 4 + b, minlength=cfg.NTILE * 4).reshape(cfg.NTILE, 4)
        per_edge.append((sc, dl, wc, t, b))

    K = np.zeros((cfg.NTILE, 4), np.int64)
    np.maximum.reduce(cnts, axis=0, out=K)
    # groups padded to 64 (PE matmul base partition must be 0/32/64, so
    # chunk pieces may only split at the 64-row midpoint); each (S,b)
    # gather call is padded to 128
    K = (K + 63) // 64 * 64

    group_base = np.zeros((cfg.NTILE, 4), np.int64)
    calls = []  # (S, b, s0, s1)
    st_tiles = []  # per S: list of (t, sl, nt)
    off = 0
    for S in range(cfg.NST):
        ts = list(range(S * G, min((S + 1) * G, cfg.NTILE)))
        st_tiles.append([(t, t - S * G, cfg.tile_nt[t]) for t in ts])
        for b in range(4):
            s0 = off
            for t in ts:
                group_base[t, b] = off
                off += K[t, b]
            off = _ru128(off)
            if off > s0:
                calls.append((S, b, s0, off))
    NSLOT = off
    NCHUNK = NSLOT // 128

    # per-chunk pieces: chunk k -> list of [sl, r0, r1, start, stop]
    chunk_b = np.zeros(NCHUNK, np.int64)
    for S, b, s0, s1 in calls:
        chunk_b[s0 // 128:s1 // 128] = b
    chunk_pieces = {}
    first_piece = {}
    last_piece = {}
    for S in range(cfg.NST):
        for b in range(4):
            for t, sl, nt in st_tiles[S]:
                kk = K[t, b]
                if kk == 0:
                    continue
                g0 = group_base[t, b]
                g1 = g0 + kk
                for k in range(g0 // 128, (g1 - 1) // 128 + 1):
                    r0 = max(g0, k * 128) - k * 128
                    r1 = min(g1, (k + 1) * 128) - k * 128
                    piece = [sl, r0, r1, False, False]
                    chunk_pieces.setdefault(k, []).append(piece)
                    if (S, sl) not in first_piece:
                        first_piece[(S, sl)] = piece
                    last_piece[(S, sl)] = piece
    for S in range(cfg.NST):
        for t, sl, nt in st_tiles[S]:
            assert (S, sl) in first_piece, f"tile {t} has no edges"
            first_piece[(S, sl)][3] = True
            last_piece[(S, sl)][4] = True

    meta = dict(NSLOT=NSLOT, NCHUNK=NCHUNK, calls=calls, st_tiles=st_tiles,
                chunk_b=chunk_b, chunk_pieces=chunk_pieces,
                Cmax=max((s1 - s0) // 128 for _, _, s0, s1 in calls))

    per_core = []
    for c in range(cfg.CORES):
        sc, dl, wc, t, b = per_edge[c]
        g = t * 4 + b
        order = np.lexsort((t, b, t // G))  # sort by (S, b, t)
        gs = g[order]
        # within-group rank (groups are contiguous runs in sorted order)
        isnew = np.empty(len(gs), bool)
        isnew[0] = True
        isnew[1:] = gs[1:] != gs[:-1]
        run_id = np.cumsum(isnew) - 1
        run_start = np.flatnonzero(isnew)
        rank = np.arange(len(gs)) - run_start[run_id]
        slot = group_base.reshape(-1)[gs] + rank
        idx_arr = np.zeros(NSLOT, np.int16)
        w_arr = np.zeros(NSLOT, np.float32)
        d_arr = np.zeros(NSLOT, np.float32)
        so, dlo, wo, to, bo = sc[order], dl[order], wc[order], t[order], b[order]
        idx_arr[slot] = (so - bo * cfg.BUCKET).astype(np.int16)
        w_arr[slot] = wo
        d_arr[slot] = (dlo - to * 128).astype(np.float32)
        idx_wrapped = np.tile(np.ascontiguousarray(idx_arr.reshape(-1, 16).T), (8, 1))
        per_core.append(dict(
            idx=idx_wrapped,
            dstl=np.ascontiguousarray(d_arr.reshape(NCHUNK, 128).T),
            wcol=np.ascontiguousarray(w_arr.reshape(NCHUNK, 128).T),
        ))
    return meta, per_core


def build_nc(cfg, meta, single_core=False):
    nc = bacc.Bacc("TRN2", target_bir_lowering=False, debug=False,
                   num_devices=1 if single_core else cfg.CORES)
    P, NTILE, NH, NCL, G = cfg.P, cfg.NTILE, cfg.NH, cfg.NCL, cfg.G
    NCHUNK, NSLOT = meta["NCHUNK"], meta["NSLOT"]

    # I/O
    xT_d = nc.dram_tensor("xT", [cfg.NF, P], BF16, kind="ExternalInput")
    idx_d = nc.dram_tensor("idx", [128, NSLOT // 16], I16, kind="ExternalInput")
    dstl_d = nc.dram_tensor("dstl", [128, NCHUNK], F32, kind="ExternalInput")
    wcol_d = nc.dram_tensor("wcol", [128, NCHUNK], F32, kind="ExternalInput")
    w1_d = nc.dram_tensor("w1", [cfg.NF, NH], BF16, kind="ExternalInput")
    w2_d = nc.dram_tensor("w2", [NH, NH], BF16, kind="ExternalInput")
    w3_d = nc.dram_tensor("w3", [NH, NH], BF16, kind="ExternalInput")
    w4_d = nc.dram_tensor("w4", [NH, NCL], BF16, kind="ExternalInput")
    b1_d = nc.dram_tensor("b1", [NH, 1], F32, kind="ExternalInput")
    b2_d = nc.dram_tensor("b2", [NH, 1], F32, kind="ExternalInput")
    b3_d = nc.dram_tensor("b3", [NH, 1], F32, kind="ExternalInput")
    b4_d = nc.dram_tensor("b4", [128, NCL], F32, kind="ExternalInput")
    iota_d = nc.dram_tensor("iota", [128, 128], BF16, kind="ExternalInput")
    out_d = nc.dram_tensor("out", [P, NCL], F32, kind="ExternalOutput")

    # internal DRAM
    ag_in = [nc.dram_tensor(f"ag_in{l}", [P, cfg.FPAD], BF16) for l in range(4)]
    table = [nc.dram_tensor(f"table{l}", [cfg.N, cfg.FPAD], BF16,
                            addr_space="Local" if single_core else "Shared")
             for l in range(4)]

    full_t = NTILE - 1 if cfg.tile_nt[-1] != 128 else NTILE
    rg = [list(range(cfg.CORES))]

    with tile.TileContext(nc) as tc, ExitStack() as ctx:
        con = ctx.enter_context(tc.tile_pool(name="const", bufs=1))
        msgs_p = ctx.enter_context(tc.tile_pool(name="msgs", bufs=8))
        ind_p = ctx.enter_context(tc.tile_pool(name="ind", bufs=16))
        h_p = ctx.enter_context(tc.tile_pool(name="hT", bufs=8))
        scr_p = ctx.enter_context(tc.tile_pool(name="scr", bufs=4))
        scr1_p = ctx.enter_context(tc.tile_pool(name="scr1", bufs=8))
        s4_p = ctx.enter_context(tc.tile_pool(name="s4p", bufs=16))
        nm_p = ctx.enter_context(tc.tile_pool(name="nmp", bufs=16))
        # one PSUM bank per in-flight dst tile (accumulation groups own a
        # whole 2KB zero region); 7 accum banks + 1 for support matmuls
        psum_p = ctx.enter_context(tc.tile_pool(name="psA", bufs=7, space="PSUM"))
        psum2_p = ctx.enter_context(tc.tile_pool(name="psB", bufs=1, space="PSUM"))

        # persistent tiles + loads (unique tags so each gets its own slot)
        def load(shape, dt, src_ap, tag):
            t_ = con.tile(shape, dt, tag=tag, name=tag)
            nc.sync.dma_start(t_[...], src_ap)
            return t_

        xT = load([cfg.NF, P], BF16, xT_d[:, :], "xT_sb")
        idx_sb = load([128, NSLOT // 16], I16, idx_d[:, :], "idx_sb")
        dstl_sb = load([128, NCHUNK], F32, dstl_d[:, :], "dstl_sb")
        wcol_sb = load([128, NCHUNK], F32, wcol_d[:, :], "wcol_sb")
        w_sb = [load([cfg.NF, NH], BF16, w1_d[:, :], "w1_sb"),
                load([NH, NH], BF16, w2_d[:, :], "w2_sb"),
                load([NH, NH], BF16, w3_d[:, :], "w3_sb"),
                load([NH, NCL], BF16, w4_d[:, :], "w4_sb")]
        b_sb = [load([NH, 1], F32, b1_d[:, :], "b1_sb"),
                load([NH, 1], F32, b2_d[:, :], "b2_sb"),
                load([NH, 1], F32, b3_d[:, :], "b3_sb"),
                load([128, NCL], F32, b4_d[:, :], "b4_sb")]
        iota_sb = load([128, 128], BF16, iota_d[:, :], "iota_sb")

        sup = con.tile([128, NTILE, cfg.FPAD], BF16, tag="sup", name="sup")
        out_sb = con.tile([128, NTILE, NCL], F32, tag="out_sb", name="out_sb")
        nc.vector.memset(sup[:, :, :], 0.0)

        def dma_sup_range(l, t0, t1):
            # stream finished support tiles [t0, t1) to the AG input as they
            # complete so the collective can fire as soon as the layer ends
            fe = min(t1, full_t)
            if fe > t0:
                nc.sync.dma_start(
                    ag_in[l][t0 * 128:fe * 128, :].rearrange(
                        "(t p) f -> p t f", p=128),
                    sup[:, t0:fe, :])
            if t1 > full_t:
                nt = cfg.tile_nt[-1]
                nc.sync.dma_start(ag_in[l][full_t * 128:P, :],
                                  sup[0:nt, NTILE - 1, :])

        def ag_only(l):
            if single_core:
                nc.sync.dma_start(table[l][0:P, :], ag_in[l][:, :])
            else:
                nc.gpsimd.collective_compute(
                    "AllGather", mybir.AluOpType.bypass, replica_groups=rg,
                    ins=[ag_in[l].ap()], outs=[table[l].ap()])

        # prologue: support1 = x @ W1
        for t in range(NTILE):
            nt = cfg.tile_nt[t]
            p2 = psum2_p.tile([128, NH], F32)
            nc.tensor.matmul(p2[0:nt, :], lhsT=xT[:, t * 128:t * 128 + nt],
                             rhs=w_sb[0][:, :], start=True, stop=True)
            nc.vector.tensor_copy(sup[0:nt, t, 0:NH], p2[0:nt, :])
        dma_sup_range(0, 0, NTILE)
        ag_only(0)

        chunk_b = meta["chunk_b"]
        calls_by_S = {}
        for S, b, s0, s1 in meta["calls"]:
            calls_by_S.setdefault(S, []).append((b, s0, s1))

        for l in range(4):  # layer l+1; gathers read table[l]
            for S in range(cfg.NST):
                S_calls = calls_by_S[S]
                S_base = S_calls[0][1]
                S_end = S_calls[-1][2]
                mt = {}   # bucket -> (msgs tile, call base chunk)
                for b, s0, s1 in S_calls:
                    n = s1 - s0
                    msgs = msgs_p.tile([128, meta["Cmax"], cfg.FPAD], BF16,
                                       tag="msgs", name="msgs")
                    nc.gpsimd.dma_gather(
                        msgs[:, 0:n // 128, :],
                        table[l][b * cfg.BUCKET:(b + 1) * cfg.BUCKET, :],
                        idx_sb[:, s0 // 16:s1 // 16],
                        num_idxs=n, num_idxs_reg=n, elem_size=cfg.FPAD,
                        single_packet=False)
                    mt[b] = (msgs, s0 // 128)
                ps_tiles = []
                for sl in range(len(meta["st_tiles"][S])):
                    ps = psum_p.tile([64, 128] if l < 3 else [128, NCL], F32,
                                     tag="ps", name="ps")
                    ps_tiles.append(ps)
                for k in range(S_base // 128, S_end // 128):
                    pieces = meta["chunk_pieces"].get(k)
                    if not pieces:
                        continue  # pure call-tail padding
                    msgs, cb = mt[int(chunk_b[k])]
                    col = k - cb
                    ind = ind_p.tile([128, 128], BF16)
                    nc.vector.tensor_scalar(
                        ind[:, :], iota_sb[:, :], dstl_sb[:, k:k + 1],
                        wcol_sb[:, k:k + 1],
                        op0=mybir.AluOpType.is_equal, op1=mybir.AluOpType.mult)
                    for sl, r0, r1, st, sp in pieces:
                        if l < 3:
                            nc.tensor.matmul(
                                ps_tiles[sl][:, :],
                                lhsT=msgs[r0:r1, col, 0:NH], rhs=ind[r0:r1, :],
                                start=st, stop=sp)
                        else:
                            nc.tensor.matmul(
                                ps_tiles[sl][:, :],
                                lhsT=ind[r0:r1, :], rhs=msgs[r0:r1, col, 0:NCL],
                                start=st, stop=sp)
                # finalize tiles of S
                if l == 3:
                    sstile = scr1_p.tile([128, G], F32, tag="sstile", name="sstile")
                    nc.vector.memset(sstile[:, :], 1.0)
                    pend = []
                for t, sl, nt in meta["st_tiles"][S]:
                    ps = ps_tiles[sl]
                    if l < 3:
                        # relu(agg + b) on DVE (ACT is slow per-op here)
                        hT = h_p.tile([NH, 128], BF16)
                        nc.vector.tensor_scalar(
                            hT[:, 0:nt], ps[0:NH, 0:nt], b_sb[l][:, 0:1], 0.0,
                            op0=mybir.AluOpType.add, op1=mybir.AluOpType.max)
                        p2 = psum2_p.tile([128, NH if l < 2 else NCL], F32)
                        d2 = NH if l < 2 else NCL
                        nc.tensor.matmul(p2[0:nt, 0:d2], lhsT=hT[:, 0:nt],
                                         rhs=w_sb[l + 1][:, :], start=True, stop=True)
                        nc.vector.tensor_copy(sup[0:nt, t, 0:d2], p2[0:nt, 0:d2])
                    else:
                        s4 = s4_p.tile([128, NCL], F32, tag="s4", name="s4")
                        nc.vector.tensor_tensor(
                            s4[0:nt, :], ps[0:nt, :],
                            b_sb[3][0:nt, :], op=mybir.AluOpType.add)
                        negm = nm_p.tile([128, 1], F32, tag="negm", name="negm")
                        nc.vector.tensor_reduce(
                            negm[0:nt, :], s4[0:nt, :], axis=mybir.AxisListType.X,
                            op=mybir.AluOpType.max, negate=True)
                        e4 = scr_p.tile([128, NCL], F32)
                        nc.scalar.activation(
                            e4[0:nt, :], s4[0:nt, :],
                            mybir.ActivationFunctionType.Exp,
                            bias=negm[0:nt, 0:1], scale=1.0,
                            accum_out=sstile[0:nt, sl:sl + 1])
                        pend.append((t, sl, nt, s4, negm))
                if l == 3:
                    # one batched Ln per supertile instead of one per tile
                    lse_t = scr1_p.tile([128, len(pend)], F32, tag="lse", name="lse")
                    nc.scalar.activation(lse_t[:, :], sstile[:, 0:len(pend)],
                                         mybir.ActivationFunctionType.Ln)
                    for t, sl, nt, s4, negm in pend:
                        nb = scr1_p.tile([128, 1], F32, tag="nb", name="nb")
                        nc.vector.tensor_sub(nb[0:nt, :], negm[0:nt, :],
                                             lse_t[0:nt, sl:sl + 1])
                        nc.vector.tensor_scalar(
                            out_sb[0:nt, t, :], s4[0:nt, :], nb[0:nt, 0:1], None,
                            op0=mybir.AluOpType.add)
                if l < 3:
                    dma_sup_range(l + 1, S * G, min((S + 1) * G, NTILE))
            if l < 3:
                ag_only(l + 1)

        # store output
        if full_t:
            nc.sync.dma_start(
                out_d[0:full_t * 128, :].rearrange("(t p) f -> p t f", p=128),
                out_sb[:, 0:full_t, :])
        if full_t < NTILE:
            nt = cfg.tile_nt[-1]
            nc.sync.dma_start(out_d[full_t * 128:P, :], out_sb[0:nt, NTILE - 1, :])

    nc.compile()
    return nc


def make_in_maps(cfg, per_core, x, W1, b1, W2, b2, W3, b3, W4, b4):
    maps = []
    iota = np.tile(np.arange(128, dtype=BFNP), (128, 1))
    for c in range(cfg.CORES):
        pc = per_core[c]
        xb = np.ascontiguousarray(
            np.asarray(x[c * cfg.P:(c + 1) * cfg.P], np.float32).T).astype(BFNP)
        maps.append({
            "xT": xb, "idx": pc["idx"], "dstl": pc["dstl"], "wcol": pc["wcol"],
            "w1": np.asarray(W1, np.float32).astype(BFNP),
            "w2": np.asarray(W2, np.float32).astype(BFNP),
            "w3": np.asarray(W3, np.float32).astype(BFNP),
            "w4": np.asarray(W4, np.float32).astype(BFNP),
            "b1": np.asarray(b1, np.float32).reshape(-1, 1),
            "b2": np.asarray(b2, np.float32).reshape(-1, 1),
            "b3": np.asarray(b3, np.float32).reshape(-1, 1),
            "b4": np.tile(np.asarray(b4, np.float32), (128, 1)),
            "iota": iota,
        })
    return maps


_CACHE = {}


def _get_built(cfg_key, edge_src, edge_dst, edge_w):
    if cfg_key not in _CACHE:
        cfg = Cfg()
        meta, per_core = build_plan(cfg, edge_src, edge_dst, edge_w)
        nc = build_nc(cfg, meta)
        _CACHE[cfg_key] = (cfg, meta, per_core, nc)
    return _CACHE[cfg_key]


def kernel(x, edge_src, edge_dst, edge_w, W1, b1, W2, b2, W3, b3, W4, b4):
    cfg, meta, per_core, nc = _get_built("full", edge_src, edge_dst, edge_w)
    in_maps = make_in_maps(cfg, per_core, x, W1, b1, W2, b2, W3, b3, W4, b4)
    res = run_bass_kernel_spmd(
        nc, in_maps, core_ids=list(range(cfg.CORES)),
        trace=bool(int(os.environ.get("GCN_TRACE", "0"))))
    out = np.concatenate([res.results[c]["out"] for c in range(cfg.CORES)], axis=0)
    kernel._last_exec_time_ns = res.exec_time_ns
    return out.astype(np.float32)


# revision 36
# speedup vs baseline: 1.8145x; 1.8145x over previous
"""4-layer GCN on 8 Trainium2 NeuronCores (Bass/Tile).

Sharding (dst-ownership, per the hint): core c owns dst nodes
[c*P, (c+1)*P), P = N/8. Each layer:
  1. each core computes its support block  h_own @ W  on the PE (bf16),
  2. AllGather of the blocks builds the full support table
     [N, 128]bf16 in DRAM (halo exchange),
  3. per-edge src rows are fetched with dma_gather (256B rows, int16
     indices in 4 src-buckets of N/4 rows),
  4. segment-sum by dst runs on the PE as one-hot matmuls: an indicator
     [128 edges x 128 dst] tile is built per chunk on the DVE with a
     single fused tensor_scalar (is_equal then *w), then
     psum[64f, 128dst] += msgs.T @ indicator accumulates f-major,
  5. finalize: ACT relu(agg + b) emits h^T tiles that feed the next
     layer's support matmul directly (no transposes anywhere).
Layer 4 aggregates node-major (indicator as lhsT) so log_softmax
reduces along the free axis.

SPMD: one program runs on all 8 cores, so per-(tile,bucket) group
sizes are the max over cores, rounded up to 64 (the PE base-partition
quantum). Chunks of 128 edge slots may span a tile boundary at the
64-row midpoint; such chunks issue two partition-sliced matmuls that
share one indicator (dst_local values are tile-relative per slot).
Pad slots gather row 0 with w=0.
"""

import os
from contextlib import ExitStack

import numpy as np
import ml_dtypes

import concourse.bass as bass
import concourse.bacc as bacc
import concourse.tile as tile
import concourse.mybir as mybir
from concourse.bass_utils import run_bass_kernel_spmd

BF16 = mybir.dt.bfloat16
F32 = mybir.dt.float32
I16 = mybir.dt.int16
BFNP = ml_dtypes.bfloat16


class Cfg:
    def __init__(self, N=100000, CORES=8, NF=128, NH=64, NCL=40, G=7):
        self.N, self.CORES, self.NF, self.NH, self.NCL, self.G = N, CORES, NF, NH, NCL, G
        assert N % CORES == 0 and N % 4 == 0
        self.P = N // CORES
        self.BUCKET = N // 4
        assert self.BUCKET <= 32767
        self.NTILE = -(-self.P // 128)
        self.tile_nt = [128] * (self.NTILE - 1) + [self.P - 128 * (self.NTILE - 1)]
        self.NST = -(-self.NTILE // G)
        self.FPAD = 128  # padded feature dim of the gather table (256B rows)


def _ru128(x):
    return (x + 127) // 128 * 128


def build_plan(cfg, edge_src, edge_dst, edge_w):
    """Host-side preprocessing. Returns (meta, per_core) where meta is
    core-independent program structure and per_core holds input arrays."""
    N, P, G = cfg.N, cfg.P, cfg.G
    src = np.asarray(edge_src, np.int64)
    dst = np.asarray(edge_dst, np.int64)
    w = np.asarray(edge_w, np.float32)

    owner = dst // P
    per_edge = []
    cnts = np.zeros((cfg.CORES, cfg.NTILE, 4), np.int64)
    for c in range(cfg.CORES):
        m = owner == c
        sc, dl, wc = src[m], dst[m] - c * P, w[m]
        t = dl >> 7
        b = sc // cfg.BUCKET
        cnts[c] = np.bincount(t * 4 + b, minlength=cfg.NTILE * 4).reshape(cfg.NTILE, 4)
        per_edge.append((sc, dl, wc, t, b))

    K = np.zeros((cfg.NTILE, 4), np.int64)
    np.maximum.reduce(cnts, axis=0, out=K)
    # groups padded to 64 (PE matmul base partition must be 0/32/64, so
    # chunk pieces may only split at the 64-row midpoint); each (S,b)
    # gather call is padded to 128
    K = (K + 63) // 64 * 64

    group_base = np.zeros((cfg.NTILE, 4), np.int64)
    calls = []  # (S, b, s0, s1)
    st_tiles = []  # per S: list of (t, sl, nt)
    off = 0
    for S in range(cfg.NST):
        ts = list(range(S * G, min((S + 1) * G, cfg.NTILE)))
        st_tiles.append([(t, t - S * G, cfg.tile_nt[t]) for t in ts])
        for b in range(4):
            s0 = off
            for t in ts:
                group_base[t, b] = off
                off += K[t, b]
            off = _ru128(off)
            if off > s0:
                calls.append((S, b, s0, off))
    NSLOT = off
    NCHUNK = NSLOT // 128

    # per-chunk pieces: chunk k -> list of [sl, r0, r1, start, stop]
    chunk_b = np.zeros(NCHUNK, np.int64)
    for S, b, s0, s1 in calls:
        chunk_b[s0 // 128:s1 // 128] = b
    chunk_pieces = {}
    first_piece = {}
    last_piece = {}
    for S in range(cfg.NST):
        for b in range(4):
            for t, sl, nt in st_tiles[S]:
                kk = K[t, b]
                if kk == 0:
                    continue
                g0 = group_base[t, b]
                g1 = g0 + kk
                for k in range(g0 // 128, (g1 - 1) // 128 + 1):
                    r0 = max(g0, k * 128) - k * 128
                    r1 = min(g1, (k + 1) * 128) - k * 128
                    piece = [sl, r0, r1, False, False]
                    chunk_pieces.setdefault(k, []).append(piece)
                    if (S, sl) not in first_piece:
                        first_piece[(S, sl)] = piece
                    last_piece[(S, sl)] = piece
    for S in range(cfg.NST):
        for t, sl, nt in st_tiles[S]:
            assert (S, sl) in first_piece, f"tile {t} has no edges"
            first_piece[(S, sl)][3] = True
            last_piece[(S, sl)][4] = True

    meta = dict(NSLOT=NSLOT, NCHUNK=NCHUNK, calls=calls, st_tiles=st_tiles,
                chunk_b=chunk_b, chunk_pieces=chunk_pieces,
                Cmax=max((s1 - s0) // 128 for _, _, s0, s1 in calls))

    per_core = []
    for c in range(cfg.CORES):
        sc, dl, wc, t, b = per_edge[c]
        g = t * 4 + b
        order = np.lexsort((t, b, t // G))  # sort by (S, b, t)
        gs = g[order]
        # within-group rank (groups are contiguous runs in sorted order)
        isnew = np.empty(len(gs), bool)
        isnew[0] = True
        isnew[1:] = gs[1:] != gs[:-1]
        run_id = np.cumsum(isnew) - 1
        run_start = np.flatnonzero(isnew)
        rank = np.arange(len(gs)) - run_start[run_id]
        slot = group_base.reshape(-1)[gs] + rank
        idx_arr = np.zeros(NSLOT, np.int16)
        w_arr = np.zeros(NSLOT, np.float32)
        d_arr = np.zeros(NSLOT, np.float32)
        so, dlo, wo, to, bo = sc[order], dl[order], wc[order], t[order], b[order]
        idx_arr[slot] = (so - bo * cfg.BUCKET).astype(np.int16)
        w_arr[slot] = wo
        d_arr[slot] = (dlo - to * 128).astype(np.float32)
        idx_wrapped = np.tile(np.ascontiguousarray(idx_arr.reshape(-1, 16).T), (8, 1))
        per_core.append(dict(
            idx=idx_wrapped,
            dstl=np.ascontiguousarray(d_arr.reshape(NCHUNK, 128).T),
            wcol=np.ascontiguousarray(w_arr.reshape(NCHUNK, 128).T),
        ))
    return meta, per_core


def build_nc(cfg, meta, single_core=False):
    nc = bacc.Bacc("TRN2", target_bir_lowering=False, debug=False,
                   num_devices=1 if single_core else cfg.CORES)
    P, NTILE, NH, NCL, G = cfg.P, cfg.NTILE, cfg.NH, cfg.NCL, cfg.G
    NCHUNK, NSLOT = meta["NCHUNK"], meta["NSLOT"]

    # I/O
    xT_d = nc.dram_tensor("xT", [cfg.NF, P], BF16, kind="ExternalInput")
    idx_d = nc.dram_tensor("idx", [128, NSLOT // 16], I16, kind="ExternalInput")
    dstl_d = nc.dram_tensor("dstl", [128, NCHUNK], F32, kind="ExternalInput")
    wcol_d = nc.dram_tensor("wcol", [128, NCHUNK], F32, kind="ExternalInput")
    w1_d = nc.dram_tensor("w1", [cfg.NF, NH], BF16, kind="ExternalInput")
    w2_d = nc.dram_tensor("w2", [NH, NH], BF16, kind="ExternalInput")
    w3_d = nc.dram_tensor("w3", [NH, NH], BF16, kind="ExternalInput")
    w4_d = nc.dram_tensor("w4", [NH, NCL], BF16, kind="ExternalInput")
    b1_d = nc.dram_tensor("b1", [NH, 1], F32, kind="ExternalInput")
    b2_d = nc.dram_tensor("b2", [NH, 1], F32, kind="ExternalInput")
    b3_d = nc.dram_tensor("b3", [NH, 1], F32, kind="ExternalInput")
    b4_d = nc.dram_tensor("b4", [128, NCL], F32, kind="ExternalInput")
    iota_d = nc.dram_tensor("iota", [128, 128], BF16, kind="ExternalInput")
    out_d = nc.dram_tensor("out", [P, NCL], F32, kind="ExternalOutput")

    # internal DRAM
    ag_in = [nc.dram_tensor(f"ag_in{l}", [P, cfg.FPAD], BF16) for l in range(4)]
    table = [nc.dram_tensor(f"table{l}", [cfg.N, cfg.FPAD], BF16,
                            addr_space="Local" if single_core else "Shared")
             for l in range(4)]

    full_t = NTILE - 1 if cfg.tile_nt[-1] != 128 else NTILE
    rg = [list(range(cfg.CORES))]

    with tile.TileContext(nc) as tc, ExitStack() as ctx:
        con = ctx.enter_context(tc.tile_pool(name="const", bufs=1))
        msgs_p = ctx.enter_context(tc.tile_pool(name="msgs", bufs=8))
        ind_p = ctx.enter_context(tc.tile_pool(name="ind", bufs=16))
        h_p = ctx.enter_context(tc.tile_pool(name="hT", bufs=8))
        scr_p = ctx.enter_context(tc.tile_pool(name="scr", bufs=4))
        scr1_p = ctx.enter_context(tc.tile_pool(name="scr1", bufs=8))
        s4_p = ctx.enter_context(tc.tile_pool(name="s4p", bufs=16))
        nm_p = ctx.enter_context(tc.tile_pool(name="nmp", bufs=16))
        # one PSUM bank per in-flight dst tile (accumulation groups own a
        # whole 2KB zero region); 7 accum banks + 1 for support matmuls
        psum_p = ctx.enter_context(tc.tile_pool(name="psA", bufs=7, space="PSUM"))
        psum2_p = ctx.enter_context(tc.tile_pool(name="psB", bufs=1, space="PSUM"))

        # persistent tiles + loads (unique tags so each gets its own slot)
        def load(shape, dt, src_ap, tag):
            t_ = con.tile(shape, dt, tag=tag, name=tag)
            nc.sync.dma_start(t_[...], src_ap)
            return t_

        xT = load([cfg.NF, P], BF16, xT_d[:, :], "xT_sb")
        idx_sb = load([128, NSLOT // 16], I16, idx_d[:, :], "idx_sb")
        dstl_sb = load([128, NCHUNK], F32, dstl_d[:, :], "dstl_sb")
        wcol_sb = load([128, NCHUNK], F32, wcol_d[:, :], "wcol_sb")
        w_sb = [load([cfg.NF, NH], BF16, w1_d[:, :], "w1_sb"),
                load([NH, NH], BF16, w2_d[:, :], "w2_sb"),
                load([NH, NH], BF16, w3_d[:, :], "w3_sb"),
                load([NH, NCL], BF16, w4_d[:, :], "w4_sb")]
        b_sb = [load([NH, 1], F32, b1_d[:, :], "b1_sb"),
                load([NH, 1], F32, b2_d[:, :], "b2_sb"),
                load([NH, 1], F32, b3_d[:, :], "b3_sb"),
                load([128, NCL], F32, b4_d[:, :], "b4_sb")]
        iota_sb = load([128, 128], BF16, iota_d[:, :], "iota_sb")

        sup = con.tile([128, NTILE, cfg.FPAD], BF16, tag="sup", name="sup")
        out_sb = con.tile([128, NTILE, NCL], F32, tag="out_sb", name="out_sb")
        nc.vector.memset(sup[:, :, :], 0.0)

        def dma_sup_range(l, t0, t1):
            # stream finished support tiles [t0, t1) to the AG input as they
            # complete so the collective can fire as soon as the layer ends
            fe = min(t1, full_t)
            if fe > t0:
                nc.sync.dma_start(
                    ag_in[l][t0 * 128:fe * 128, :].rearrange(
                        "(t p) f -> p t f", p=128),
                    sup[:, t0:fe, :])
            if t1 > full_t:
                nt = cfg.tile_nt[-1]
                nc.sync.dma_start(ag_in[l][full_t * 128:P, :],
                                  sup[0:nt, NTILE - 1, :])

        def ag_only(l):
            if single_core:
                nc.sync.dma_start(table[l][0:P, :], ag_in[l][:, :])
            else:
                nc.gpsimd.collective_compute(
                    "AllGather", mybir.AluOpType.bypass, replica_groups=rg,
                    ins=[ag_in[l].ap()], outs=[table[l].ap()])

        # prologue: support1 = x @ W1
        for t in range(NTILE):
            nt = cfg.tile_nt[t]
            p2 = psum2_p.tile([128, NH], F32)
            nc.tensor.matmul(p2[0:nt, :], lhsT=xT[:, t * 128:t * 128 + nt],
                             rhs=w_sb[0][:, :], start=True, stop=True)
            nc.vector.tensor_copy(sup[0:nt, t, 0:NH], p2[0:nt, :])
        dma_sup_range(0, 0, NTILE)
        ag_only(0)

        chunk_b = meta["chunk_b"]
        calls_by_S = {}
        for S, b, s0, s1 in meta["calls"]:
            calls_by_S.setdefault(S, []).append((b, s0, s1))

        for l in range(4):  # layer l+1; gathers read table[l]
            for S in range(cfg.NST):
                S_calls = calls_by_S[S]
                S_base = S_calls[0][1]
                S_end = S_calls[-1][2]
                mt = {}   # bucket -> (msgs tile, call base chunk)
                for b, s0, s1 in S_calls:
                    n = s1 - s0
                    msgs = msgs_p.tile([128, meta["Cmax"], cfg.FPAD], BF16,
                                       tag="msgs", name="msgs")
                    nc.gpsimd.dma_gather(
                        msgs[:, 0:n // 128, :],
                        table[l][b * cfg.BUCKET:(b + 1) * cfg.BUCKET, :],
                        idx_sb[:, s0 // 16:s1 // 16],
                        num_idxs=n, num_idxs_reg=n, elem_size=cfg.FPAD,
                        single_packet=False)
                    mt[b] = (msgs, s0 // 128)
                ps_tiles = []
                for sl in range(len(meta["st_tiles"][S])):
                    ps = psum_p.tile([64, 128] if l < 3 else [128, NCL], F32,
                                     tag="ps", name="ps")
                    ps_tiles.append(ps)
                for k in range(S_base // 128, S_end // 128):
                    pieces = meta["chunk_pieces"].get(k)
                    if not pieces:
                        continue  # pure call-tail padding
                    msgs, cb = mt[int(chunk_b[k])]
                    col = k - cb
                    ind = ind_p.tile([128, 128], BF16)
                    nc.vector.tensor_scalar(
                        ind[:, :], iota_sb[:, :], dstl_sb[:, k:k + 1],
                        wcol_sb[:, k:k + 1],
                        op0=mybir.AluOpType.is_equal, op1=mybir.AluOpType.mult)
                    for sl, r0, r1, st, sp in pieces:
                        if l < 3:
                            nc.tensor.matmul(
                                ps_tiles[sl][:, :],
                                lhsT=msgs[r0:r1, col, 0:NH], rhs=ind[r0:r1, :],
                                start=st, stop=sp)
                        else:
                            nc.tensor.matmul(
                                ps_tiles[sl][:, :],
                                lhsT=ind[r0:r1, :], rhs=msgs[r0:r1, col, 0:NCL],
                                start=st, stop=sp)
                # finalize tiles of S
                if l == 3:
                    sstile = scr1_p.tile([128, G], F32, tag="sstile", name="sstile")
                    nc.vector.memset(sstile[:, :], 1.0)
                    pend = []
                for t, sl, nt in meta["st_tiles"][S]:
                    ps = ps_tiles[sl]
                    if l < 3:
                        # relu(agg + b) on DVE (ACT is slow per-op here)
                        hT = h_p.tile([NH, 128], BF16)
                        nc.vector.tensor_scalar(
                            hT[:, 0:nt], ps[0:NH, 0:nt], b_sb[l][:, 0:1], 0.0,
                            op0=mybir.AluOpType.add, op1=mybir.AluOpType.max)
                        p2 = psum2_p.tile([128, NH if l < 2 else NCL], F32)
                        d2 = NH if l < 2 else NCL
                        nc.tensor.matmul(p2[0:nt, 0:d2], lhsT=hT[:, 0:nt],
                                         rhs=w_sb[l + 1][:, :], start=True, stop=True)
                        nc.vector.tensor_copy(sup[0:nt, t, 0:d2], p2[0:nt, 0:d2])
                    else:
                        s4 = s4_p.tile([128, NCL], F32, tag="s4", name="s4")
                        nc.vector.tensor_tensor(
                            s4[0:nt, :], ps[0:nt, :],
                            b_sb[3][0:nt, :], op=mybir.AluOpType.add)
                        negm = nm_p.tile([128, 1], F32, tag="negm", name="negm")
                        nc.vector.tensor_reduce(
                            negm[0:nt, :], s4[0:nt, :], axis=mybir.AxisListType.X,
                            op=mybir.AluOpType.max, negate=True)
                        e4 = scr_p.tile([128, NCL], F32)
                        nc.scalar.activation(
                            e4[0:nt, :], s4[0:nt, :],
                            mybir.ActivationFunctionType.Exp,
                            bias=negm[0:nt, 0:1], scale=1.0,
                            accum_out=sstile[0:nt, sl:sl + 1])
                        pend.append((t, sl, nt, s4, negm))
                if l == 3:
                    # one batched Ln per supertile instead of one per tile
                    lse_t = scr1_p.tile([128, len(pend)], F32, tag="lse", name="lse")
                    nc.scalar.activation(lse_t[:, :], sstile[:, 0:len(pend)],
                                         mybir.ActivationFunctionType.Ln)
                    for t, sl, nt, s4, negm in pend:
                        nb = scr1_p.tile([128, 1], F32, tag="nb", name="nb")
                        nc.vector.tensor_sub(nb[0:nt, :], negm[0:nt, :],
                                             lse_t[0:nt, sl:sl + 1])
                        nc.vector.tensor_scalar(
                            out_sb[0:nt, t, :], s4[0:nt, :], nb[0:nt, 0:1], None,
                            op0=mybir.AluOpType.add)
                if l < 3:
                    dma_sup_range(l + 1, S * G, min((S + 1) * G, NTILE))
            if l < 3:
                ag_only(l + 1)

        # store output
        if full_t:
            nc.sync.dma_start(
                out_d[0:full_t * 128, :].rearrange("(t p) f -> p t f", p=128),
                out_sb[:, 0:full_t, :])
        if full_t < NTILE:
            nt = cfg.tile_nt[-1]
            nc.sync.dma_start(out_d[full_t * 128:P, :], out_sb[0:nt, NTILE - 1, :])

    nc.compile()
    return nc


def make_in_maps(cfg, per_core, x, W1, b1, W2, b2, W3, b3, W4, b4):
    maps = []
    iota = np.tile(np.arange(128, dtype=BFNP), (128, 1))
    for c in range(cfg.CORES):
        pc = per_core[c]
        xb = np.ascontiguousarray(
            np.asarray(x[c * cfg.P:(c + 1) * cfg.P], np.float32).T).astype(BFNP)
        maps.append({
            "xT": xb, "idx": pc["idx"], "dstl": pc["dstl"], "wcol": pc["wcol"],
            "w1": np.asarray(W1, np.float32).astype(BFNP),
            "w2": np.asarray(W2, np.float32).astype(BFNP),
            "w3": np.asarray(W3, np.float32).astype(BFNP),
            "w4": np.asarray(W4, np.float32).astype(BFNP),
            "b1": np.asarray(b1, np.float32).reshape(-1, 1),
            "b2": np.asarray(b2, np.float32).reshape(-1, 1),
            "b3": np.asarray(b3, np.float32).reshape(-1, 1),
            "b4": np.tile(np.asarray(b4, np.float32), (128, 1)),
            "iota": iota,
        })
    return maps


def balance_perm(cfg, edge_src, edge_dst):
    """Quarter-preserving node permutation balancing per-(tile,bucket) edge
    counts across cores, to shrink the SPMD max-over-cores padding.
    Position p holds original node perm[p]; src buckets are unchanged
    because positions stay within the node's original quarter."""
    N, B, P = cfg.N, cfg.BUCKET, cfg.P
    src = np.asarray(edge_src, np.int64)
    dst = np.asarray(edge_dst, np.int64)
    d = np.bincount(dst * 4 + src // B, minlength=N * 4).reshape(N, 4)
    Q = N // 4
    assert Q == B and P * 2 == Q, "quarter layout requires 8 cores, 4 buckets"
    perm = np.empty(N, np.int64)
    for q in range(4):
        nodes = np.arange(q * Q, (q + 1) * Q)
        dq = d[nodes]
        order = np.argsort(-dq.sum(1), kind="stable")
        nodes, dq = nodes[order], dq[order]
        bins = [(c, t) for c in (2 * q, 2 * q + 1) for t in range(cfg.NTILE)]
        cap = np.array([cfg.tile_nt[t] for _, t in bins])
        L = np.zeros((len(bins), 4), np.int64)
        cnt = np.zeros(len(bins), np.int64)
        assign = [[] for _ in bins]
        for i in range(len(nodes)):
            score = (L + dq[i]).max(axis=1).astype(np.float64)
            score[cnt >= cap] = np.inf
            j = int(np.argmin(score))
            assign[j].append(i)
            L[j] += dq[i]
            cnt[j] += 1
        for j, (c, t) in enumerate(bins):
            ids = nodes[assign[j]]
            pos0 = c * P + t * 128
            perm[pos0:pos0 + len(ids)] = ids
    pinv = np.empty(N, np.int64)
    pinv[perm] = np.arange(N)
    return perm, pinv


_CACHE = {}


def _get_built(cfg_key, edge_src, edge_dst, edge_w):
    if cfg_key not in _CACHE:
        cfg = Cfg()
        perm, pinv = balance_perm(cfg, edge_src, edge_dst)
        src_pos = pinv[np.asarray(edge_src, np.int64)]
        dst_pos = pinv[np.asarray(edge_dst, np.int64)]
        meta, per_core = build_plan(cfg, src_pos, dst_pos, edge_w)
        nc = build_nc(cfg, meta)
        _CACHE[cfg_key] = (cfg, meta, per_core, nc, perm)
    return _CACHE[cfg_key]


def kernel(x, edge_src, edge_dst, edge_w, W1, b1, W2, b2, W3, b3, W4, b4):
    cfg, meta, per_core, nc, perm = _get_built("full", edge_src, edge_dst, edge_w)
    x_perm = np.asarray(x, np.float32)[perm]
    in_maps = make_in_maps(cfg, per_core, x_perm, W1, b1, W2, b2, W3, b3, W4, b4)
    res = run_bass_kernel_spmd(
        nc, in_maps, core_ids=list(range(cfg.CORES)),
        trace=bool(int(os.environ.get("GCN_TRACE", "0"))))
    pos_out = np.concatenate([res.results[c]["out"] for c in range(cfg.CORES)],
                             axis=0)
    out = np.empty_like(pos_out)
    out[perm] = pos_out
    kernel._last_exec_time_ns = res.exec_time_ns
    return out.astype(np.float32)
